# revision 11
# baseline (speedup 1.0000x reference)
"""Trainium2 Bass kernel for nn_RecurrentGCN (TGCN cell + MLP head, output = y[2]).

The reference network returns y[2] — a single [1]-shaped value that depends only
on node 2's GCN aggregation.  With H0 = 0 the r-gate branch (Wr/br/Lr_*) and the
bottom halves of Lz_W/Lh_W are multiplied by zero, so the live computation is:

    deg[n]   = 1 + #(dst == n)                     (self loops add 1)
    g        = dinv2 * ( sum_{e: dst[e]==2} dinv[src[e]] * x[src[e]]
                         + dinv2 * x[2] )          with dinv = rsqrt(deg)
    cz = g @ Wz + bz ;  ch = g @ Wh + bh
    Z  = sigmoid(cz @ Lz_W[:64] + Lz_b) ; Ht = tanh(ch @ Lh_W[:64] + Lh_b)
    h  = (1 - Z) * Ht
    y  = relu(h) @ W1 + b1  -> BN(eval) -> relu -> @ W2 + b2

Sharding: the 1.6M-entry dst array (the memory-bound part) is sharded across the
8 NeuronCores; each core counts the occurrences of the candidate node set (node 2
plus the unique sources of its in-edges) over its shard with DVE is_equal +
accumulate ops and ACT square/relu indicator ops.  Partial counts are summed
with an on-chip AllReduce, after which every core computes the (tiny) dense
epilogue; core 0's output is returned.  Host work is limited to locating node
2's in-edges, packing parameters, and padding/sharding dst.
"""

import numpy as np

DEBUG = False

N = 100000
E = 1600000
HD = 64
BN_EPS = 1e-5
NCORES = 8
PART = 128
FREE = 1564                      # 128*1564 = 200192 >= E/8, per-core shard
SHARD = PART * FREE
PAD_DST = -1.0                   # never equals a real node id or candidate


def _build_program(u_pad, n_dve):
    """Build the SPMD Bass program for u_pad candidate slots.

    Slots [0, n_dve) are counted on the DVE engine (1 op/slot), the rest on
    the ACT engine (2 ops/slot).
    """
    import concourse.bass as bass
    import concourse.mybir as mybir

    AF = mybir.ActivationFunctionType
    ALU = mybir.AluOpType

    # ---- parameter pack layout (one [128, PF] f32 tensor) ----
    C_ONES = 0          # 1.0 in all 128 rows (matmul rhs for partition reduce)
    C_BZ = 1            # bz (rows 0:64)
    C_BH = 2            # bh (rows 0:64)
    C_LZB = 3           # Lz_b (rows 0:64)
    C_LHB = 4           # Lh_b (rows 0:64)
    C_B1 = 5            # b1 (rows 0:64)
    C_RM = 6            # rmean (rows 0:64)
    C_BNG = 7           # gamma * rsqrt(rvar+eps) (rows 0:64)
    C_BETA = 8          # beta (rows 0:64)
    C_B2 = 9            # b2 (row 0)
    C_MULT = 10         # candidate multiplicity weights (rows 0:u_pad)
    C_ROW1 = 11                  # ones_row: 1.0s in row 0, 128 cols
    C_CAND = C_ROW1 + 128        # candidate ids, row 0, u_pad cols
    C_NCAND = C_CAND + u_pad     # negated candidate ids, row 0, u_pad cols
    C_XG = C_NCAND + u_pad       # x rows of candidates [u_pad, 64]
    C_WZ = C_XG + 64             # Wz [64, 64]
    C_WH = C_WZ + 64             # Wh [64, 64]
    C_LZ = C_WH + 64             # Lz_W[:64] [64, 64]
    C_LH = C_LZ + 64             # Lh_W[:64] [64, 64]
    C_W1 = C_LH + 64             # W1 [64, 64]
    C_W2 = C_W1 + 64             # W2 [64, 1]
    PF = C_W2 + 1

    nc = bass.Bass()
    f32 = mybir.dt.float32

    dstv = nc.declare_dram_parameter("dstv", [PART, FREE], f32, isOutput=False)
    pp = nc.declare_dram_parameter("pp", [PART, PF], f32, isOutput=False)
    out = nc.declare_dram_parameter("out", [1, 1], f32, isOutput=True)
    if DEBUG:
        dbg = nc.declare_dram_parameter("dbg", [PART, 16], f32, isOutput=True)
        dbg2 = nc.declare_dram_parameter("dbg2", [PART, 2 * u_pad], f32, isOutput=True)

    cc_in = nc.dram_tensor("cc_in", [u_pad, 1], f32)
    cc_out = nc.dram_tensor("cc_out", [u_pad, 1], f32)

    dve_set = list(range(n_dve))
    act_set = list(range(n_dve, u_pad))

    from contextlib import ExitStack

    with ExitStack() as ctx:
        ec = ctx.enter_context
        dst_t = ec(nc.sbuf_tensor("dst_t", [PART, FREE], f32))
        scr = ec(nc.sbuf_tensor("scr", [PART, FREE], f32))
        usq = ec(nc.sbuf_tensor("usq", [PART, FREE], f32))
        ind = ec(nc.sbuf_tensor("ind", [PART, FREE], f32))
        scr2 = ec(nc.sbuf_tensor("scr2", [PART, FREE], f32))
        p_sb = ec(nc.sbuf_tensor("p_sb", [PART, PF], f32))
        cbc = ec(nc.sbuf_tensor("cbc", [PART, 2 * u_pad], f32))
        cntp = ec(nc.sbuf_tensor("cntp", [PART, u_pad], f32))
        cnt_loc = ec(nc.sbuf_tensor("cnt_loc", [u_pad, 1], f32))
        cnt_tot = ec(nc.sbuf_tensor("cnt_tot", [u_pad, 1], f32))
        s_deg = ec(nc.sbuf_tensor("s_deg", [u_pad, 1], f32))
        dinv = ec(nc.sbuf_tensor("dinv", [u_pad, 1], f32))
        w1c = ec(nc.sbuf_tensor("w1c", [u_pad, 1], f32))
        w_col = ec(nc.sbuf_tensor("w_col", [u_pad, 1], f32))
        g_sb = ec(nc.sbuf_tensor("g_sb", [HD, 1], f32))
        cz_sb = ec(nc.sbuf_tensor("cz_sb", [HD, 1], f32))
        ch_sb = ec(nc.sbuf_tensor("ch_sb", [HD, 1], f32))
        z_sb = ec(nc.sbuf_tensor("z_sb", [HD, 1], f32))
        ht_sb = ec(nc.sbuf_tensor("ht_sb", [HD, 1], f32))
        zm_sb = ec(nc.sbuf_tensor("zm_sb", [HD, 1], f32))
        h_sb = ec(nc.sbuf_tensor("h_sb", [HD, 1], f32))
        y_sb = ec(nc.sbuf_tensor("y_sb", [HD, 1], f32))
        y1_sb = ec(nc.sbuf_tensor("y1_sb", [HD, 1], f32))
        t_sb = ec(nc.sbuf_tensor("t_sb", [HD, 1], f32))
        t2_sb = ec(nc.sbuf_tensor("t2_sb", [HD, 1], f32))
        yr_sb = ec(nc.sbuf_tensor("yr_sb", [HD, 1], f32))
        o_sb = ec(nc.sbuf_tensor("o_sb", [1, 1], f32))
        # PSUM: 4 tensors (8 banks exist); reuse is serialized by the tok chain
        psA = ec(nc.psum_tensor("psA", [PART, 2 * u_pad], f32))
        psB = ec(nc.psum_tensor("psB", [PART, 1], f32))
        psC = ec(nc.psum_tensor("psC", [PART, 1], f32))
        psD = ec(nc.psum_tensor("psD", [PART, 1], f32))
        dsem = ec(nc.semaphore("dsem"))    # input DMAs (16/32)
        bsem = ec(nc.semaphore("bsem"))    # candidate broadcast (1: psum, 2: sbuf)
        csem = ec(nc.semaphore("csem"))    # count loops done (2)
        rsem = ec(nc.semaphore("rsem"))    # partition-reduce matmul done
        lsem = ec(nc.semaphore("lsem"))    # cnt_loc in sbuf
        ccs = ec(nc.semaphore("ccs"))      # collective done
        psem = ec(nc.semaphore("psem"))    # gpsimd DMA (16)
        gsem = ec(nc.semaphore("gsem"))    # cnt_tot in sbuf (16)
        tok = ec(nc.semaphore("tok"))      # epilogue chain
        osem = ec(nc.semaphore("osem"))    # output ready in sbuf
        block = ec(nc.Block())
        ps_bc = psA[:, :]                  # [128, 2u]   candidate broadcast
        ps_cnt = psB[0:u_pad, :]           # [u, 1]      summed partial counts
        ps_d0 = psC[0:u_pad, :]            # [u, 1]      dinv0 broadcast
        ps_g = psB[0:HD, :]                # [64, 1]     aggregated x row
        ps_cz = psC[0:HD, :]               # [64, 1]
        ps_ch = psD[0:HD, :]               # [64, 1]
        ps_z = psA[0:HD, 0:1]              # [64, 1]
        ps_h = psC[0:HD, :]                # [64, 1]  (ps_cz consumed by then)
        ps_y1 = psB[0:HD, :]               # [64, 1]  (ps_g consumed by then)
        ps_o = psD[0:1, :]                 # [1, 1]   (ps_ch consumed by then)

        @block.sync
        def _(sync):
            sync.dma_start(p_sb[:, :], pp[:, :]).then_inc(dsem, 16)
            sync.dma_start(dst_t[:, :], dstv[:, :]).then_inc(dsem, 16)
            sync.wait_ge(osem, 1)
            sync.dma_start(out[:, :], o_sb[:, :]).then_inc(dsem, 16)
            if DEBUG:
                with nc.allow_non_contiguous_dma(reason="debug dumps"):
                    for c, t in enumerate([
                        cnt_loc, cnt_tot, s_deg, dinv, w1c, w_col, g_sb, cz_sb,
                        ch_sb, z_sb, ht_sb, h_sb, y1_sb, t2_sb, yr_sb,
                    ]):
                        sync.dma_start(
                            dbg[0:t.shape[0], c:c + 1], t[:, :]
                        ).then_inc(dsem, 16)
                sync.dma_start(dbg2[:, 0:u_pad], cntp[:, :]).then_inc(dsem, 16)
                sync.dma_start(dbg2[:, u_pad:2 * u_pad], cbc[:, 0:u_pad]).then_inc(
                    dsem, 16
                )

        @block.tensor
        def _(pe):
            pe.wait_ge(dsem, 16)
            # broadcast candidate ids (pos | neg) to all 128 partitions
            pe.matmul(
                ps_bc,
                p_sb[0:1, C_ROW1:C_ROW1 + 128],
                p_sb[0:1, C_CAND:C_CAND + 2 * u_pad],
            ).then_inc(bsem, 1)
            pe.wait_ge(csem, 2)
            pe.matmul(ps_cnt, cntp[:, :], p_sb[:, C_ONES:C_ONES + 1]).then_inc(
                rsem, 1
            )
            # epilogue matmuls
            pe.wait_ge(tok, 2)  # dinv ready
            pe.matmul(
                ps_d0, p_sb[0:1, C_ROW1:C_ROW1 + u_pad], dinv[0:1, 0:1]
            ).then_inc(tok, 1)
            pe.wait_ge(tok, 5)
            pe.matmul(ps_g, p_sb[0:u_pad, C_XG:C_XG + HD], w_col[:, :]).then_inc(
                tok, 1
            )  # -> 6
            pe.wait_ge(tok, 7)
            pe.matmul(ps_cz, p_sb[0:HD, C_WZ:C_WZ + HD], g_sb[:, :]).then_inc(tok, 1)
            pe.matmul(ps_ch, p_sb[0:HD, C_WH:C_WH + HD], g_sb[:, :]).then_inc(tok, 1)
            pe.wait_ge(tok, 11)
            pe.matmul(ps_z, p_sb[0:HD, C_LZ:C_LZ + HD], cz_sb[:, :]).then_inc(tok, 1)
            pe.matmul(ps_h, p_sb[0:HD, C_LH:C_LH + HD], ch_sb[:, :]).then_inc(tok, 1)
            pe.wait_ge(tok, 18)
            pe.matmul(ps_y1, p_sb[0:HD, C_W1:C_W1 + HD], y_sb[:, :]).then_inc(tok, 1)
            pe.wait_ge(tok, 23)
            pe.matmul(ps_o, p_sb[0:HD, C_W2:C_W2 + 1], yr_sb[:, :]).then_inc(tok, 1)

        @block.scalar
        def _(act):
            act.wait_ge(bsem, 1)
            act.copy(cbc[:, :], ps_bc).then_inc(bsem, 1)
            # bsem>=2 implies our own cbc copy retired (ACT bias reads are
            # latched early -> same-engine RAW on bias needs a real wait)
            act.wait_ge(bsem, 2)
            act.wait_ge(dsem, 32)
            for i, j in enumerate(act_set):
                u_t = usq if i % 2 == 0 else ind  # double buffer the |d| tile
                act.activation(
                    u_t[:, :],
                    dst_t[:, :],
                    AF.Abs,
                    bias=cbc[:, u_pad + j:u_pad + j + 1],
                    scale=1.0,
                )
                last = act.activation(
                    scr2[:, :],
                    u_t[:, :],
                    AF.Relu,
                    bias=1.0,
                    scale=-1.0,
                    accum_out=cntp[:, j:j + 1],
                )
            last.then_inc(csem, 1)
            act.wait_ge(rsem, 1)
            act.copy(cnt_loc[:, :], ps_cnt).then_inc(lsem, 1)
            # ---- epilogue ----
            act.wait_ge(gsem, 16)
            act.activation(
                s_deg[:, :], cnt_tot[:, :], AF.Sqrt, bias=1.0, scale=1.0
            ).then_inc(tok, 1)  # -> 1
            act.wait_ge(tok, 6)
            act.copy(g_sb[:, :], ps_g).then_inc(tok, 1)  # -> 7
            act.wait_ge(tok, 9)
            act.activation(
                cz_sb[:, :], ps_cz, AF.Identity,
                bias=p_sb[0:HD, C_BZ:C_BZ + 1], scale=1.0,
            ).then_inc(tok, 1)  # -> 10
            act.activation(
                ch_sb[:, :], ps_ch, AF.Identity,
                bias=p_sb[0:HD, C_BH:C_BH + 1], scale=1.0,
            ).then_inc(tok, 1)  # -> 11
            act.wait_ge(tok, 13)
            act.activation(
                z_sb[:, :], ps_z, AF.Sigmoid,
                bias=p_sb[0:HD, C_LZB:C_LZB + 1], scale=1.0,
            ).then_inc(tok, 1)  # -> 14
            act.activation(
                ht_sb[:, :], ps_h, AF.Tanh,
                bias=p_sb[0:HD, C_LHB:C_LHB + 1], scale=1.0,
            ).then_inc(tok, 1)  # -> 15
            act.wait_ge(tok, 17)
            act.activation(
                y_sb[:, :], h_sb[:, :], AF.Relu, bias=0.0, scale=1.0
            ).then_inc(tok, 1)  # -> 18
            act.wait_ge(tok, 19)
            act.activation(
                y1_sb[:, :], ps_y1, AF.Identity,
                bias=p_sb[0:HD, C_B1:C_B1 + 1], scale=1.0,
            ).then_inc(tok, 1)  # -> 20
            act.wait_ge(tok, 22)
            act.activation(
                yr_sb[:, :], t2_sb[:, :], AF.Relu,
                bias=p_sb[0:HD, C_BETA:C_BETA + 1], scale=1.0,
            ).then_inc(tok, 1)  # -> 23
            act.wait_ge(tok, 24)
            act.activation(
                o_sb[:, :], ps_o, AF.Identity,
                bias=p_sb[0:1, C_B2:C_B2 + 1], scale=1.0,
            ).then_inc(osem, 1)

        @block.vector
        def _(dve):
            dve.wait_ge(bsem, 2)
            dve.wait_ge(dsem, 32)
            for j in dve_set:
                last = dve.tensor_scalar(
                    scr[:, :],
                    dst_t[:, :],
                    cbc[:, j:j + 1],
                    None,
                    ALU.is_equal,
                    ALU.add,
                    accum_out=cntp[:, j:j + 1],
                )
            last.then_inc(csem, 1)
            # ---- epilogue ----
            dve.wait_ge(tok, 1)
            dve.reciprocal(dinv[:, :], s_deg[:, :]).then_inc(tok, 1)  # -> 2
            dve.wait_ge(tok, 2)  # own-retire barrier: dinv visible
            dve.tensor_tensor(
                w1c[:, :], dinv[:, :], p_sb[0:u_pad, C_MULT:C_MULT + 1], ALU.mult
            ).then_inc(tok, 1)  # -> 3  (pe's ps_d0 makes 4)
            dve.wait_ge(tok, 4)
            dve.tensor_tensor(w_col[:, :], w1c[:, :], ps_d0, ALU.mult).then_inc(
                tok, 1
            )  # -> 5
            dve.wait_ge(tok, 15)
            dve.tensor_scalar(
                zm_sb[:, :], z_sb[:, :], -1.0, 1.0, ALU.mult, ALU.add
            ).then_inc(tok, 1)  # -> 16
            dve.wait_ge(tok, 16)  # own-retire barrier: zm visible
            dve.tensor_tensor(
                h_sb[:, :], zm_sb[:, :], ht_sb[:, :], ALU.mult
            ).then_inc(tok, 1)  # -> 17
            dve.wait_ge(tok, 20)
            dve.tensor_tensor(
                t_sb[:, :], y1_sb[:, :], p_sb[0:HD, C_RM:C_RM + 1], ALU.subtract
            ).then_inc(tok, 1)  # -> 21
            dve.wait_ge(tok, 21)  # own-retire barrier: t visible
            dve.tensor_tensor(
                t2_sb[:, :], t_sb[:, :], p_sb[0:HD, C_BNG:C_BNG + 1], ALU.mult
            ).then_inc(tok, 1)  # -> 22

        @block.gpsimd
        def _(gp):
            gp.wait_ge(lsem, 1)
            gp.dma_start(cc_in[:, :], cnt_loc[:, :]).then_inc(psem, 16)
            gp.wait_ge(psem, 16)
            gp.collective_compute(
                "AllReduce",
                mybir.AluOpType.add,
                replica_groups=[list(range(NCORES))],
                ins=[cc_in[:, :].opt()],
                outs=[cc_out[:, :].opt()],
            ).then_inc(ccs, 1)
            gp.wait_ge(ccs, 1)
            gp.dma_start(cnt_tot[:, :], cc_out[:, :]).then_inc(gsem, 16)

    layout = dict(
        C_ONES=C_ONES, C_BZ=C_BZ, C_BH=C_BH, C_LZB=C_LZB, C_LHB=C_LHB, C_B1=C_B1,
        C_RM=C_RM, C_BNG=C_BNG, C_BETA=C_BETA, C_B2=C_B2, C_MULT=C_MULT,
        C_ROW1=C_ROW1, C_CAND=C_CAND, C_NCAND=C_NCAND, C_XG=C_XG, C_WZ=C_WZ,
        C_WH=C_WH, C_LZ=C_LZ, C_LH=C_LH, C_W1=C_W1, C_W2=C_W2, PF=PF,
    )
    return nc, layout


def _prepare(inputs):
    """Host-side preprocessing: find node 2's in-edges, pack params, shard dst."""
    x = np.asarray(inputs["x"], np.float32)
    src = np.asarray(inputs["src"])
    dst = np.asarray(inputs["dst"])

    pos = np.flatnonzero(dst == 2)
    srcs = src[pos]
    uniq, mult = np.unique(srcs, return_counts=True)
    # slot 0 = node 2 itself (for deg2 / the self loop term); then unique sources
    n_slots = 1 + len(uniq)
    u_pad = max(8, -(-(n_slots + 1) // 4) * 4)
    assert n_slots <= 120, f"unexpectedly many in-edges at node 2: {n_slots}"

    cand = np.full(u_pad, -5.0, np.float32)
    multv = np.zeros(u_pad, np.float32)
    cand[0] = 2.0
    multv[0] = 1.0
    cand[1:n_slots] = uniq.astype(np.float32)
    multv[1:n_slots] = mult.astype(np.float32)

    xg = np.zeros((u_pad, HD), np.float32)
    xg[0] = x[2]
    if len(uniq):
        xg[1:n_slots] = x[uniq]

    n_dve = (2 * u_pad) // 3  # DVE slot is ~2x cheaper than ACT's 2-op slot

    nc, L = _build_program(u_pad, n_dve)

    PF = L["PF"]
    P = np.zeros((PART, PF), np.float32)
    P[:, L["C_ONES"]] = 1.0
    P[0:HD, L["C_BZ"]] = np.asarray(inputs["bz"], np.float32)
    P[0:HD, L["C_BH"]] = np.asarray(inputs["bh"], np.float32)
    P[0:HD, L["C_LZB"]] = np.asarray(inputs["Lz_b"], np.float32)
    P[0:HD, L["C_LHB"]] = np.asarray(inputs["Lh_b"], np.float32)
    P[0:HD, L["C_B1"]] = np.asarray(inputs["b1"], np.float32)
    P[0:HD, L["C_RM"]] = np.asarray(inputs["rmean"], np.float32)
    rvar = np.asarray(inputs["rvar"], np.float64)
    gamma = np.asarray(inputs["gamma"], np.float64)
    P[0:HD, L["C_BNG"]] = (gamma / np.sqrt(rvar + BN_EPS)).astype(np.float32)
    P[0:HD, L["C_BETA"]] = np.asarray(inputs["beta"], np.float32)
    P[0, L["C_B2"]] = np.asarray(inputs["b2"], np.float32)[0]
    P[0:u_pad, L["C_MULT"]] = multv
    P[0, L["C_ROW1"]:L["C_ROW1"] + 128] = 1.0
    P[0, L["C_CAND"]:L["C_CAND"] + u_pad] = cand
    P[0, L["C_NCAND"]:L["C_NCAND"] + u_pad] = -cand
    P[0:u_pad, L["C_XG"]:L["C_XG"] + HD] = xg
    P[0:HD, L["C_WZ"]:L["C_WZ"] + HD] = np.asarray(inputs["Wz"], np.float32)
    P[0:HD, L["C_WH"]:L["C_WH"] + HD] = np.asarray(inputs["Wh"], np.float32)
    P[0:HD, L["C_LZ"]:L["C_LZ"] + HD] = np.asarray(inputs["Lz_W"], np.float32)[:HD]
    P[0:HD, L["C_LH"]:L["C_LH"] + HD] = np.asarray(inputs["Lh_W"], np.float32)[:HD]
    P[0:HD, L["C_W1"]:L["C_W1"] + HD] = np.asarray(inputs["W1"], np.float32)
    P[0:HD, L["C_W2"]] = np.asarray(inputs["W2"], np.float32)[:, 0]

    dstp = np.full(NCORES * SHARD, PAD_DST, np.float32)
    dstp[:E] = dst.astype(np.float32)
    shards = dstp.reshape(NCORES, PART, FREE)

    in_maps = [{"dstv": shards[i], "pp": P} for i in range(NCORES)]
    return nc, in_maps


def _run(inputs, trace=False):
    from concourse.bass_utils import run_bass_kernel_spmd

    nc, in_maps = _prepare(inputs)
    res = run_bass_kernel_spmd(
        nc, in_maps, core_ids=list(range(NCORES)), trace=trace
    )
    out = np.asarray(res.results[0]["out"], np.float32).reshape(1)
    return out, res


def kernel(**inputs):
    out, _ = _run(inputs, trace=False)
    return out
